# revision 18
# baseline (speedup 1.0000x reference)
"""Causal self-attention (B=2, T=2048, D=2048, H=16, d=128) on 8 TRN2 NeuronCores.

Sharding: head-parallel compute, token-parallel output. Core c owns heads
{2c, 2c+1} for both batches: column-parallel QKV projection, per-head RoPE +
causal attention. The per-head attention outputs are exchanged with a single
AllToAll per batch (each core sends each peer the 256 token rows that peer
owns), after which every core holds all 16 heads for its own 256 rows and
computes the full output projection locally — no reduction collective needed.
Host concatenates the 8 contiguous row shards.

Host-prepped layouts (sharding/layout prep only — all math on device):
  xT      [2, D, T]    x transposed per batch (bf16)
  wqk     [D, 512]     qkv_w rows [q_h0,q_h1,k_h0,k_h1] transposed (bf16)
  wv      [D, 256]     qkv_w v rows transposed (bf16)
  wo      [D, D]       full out_w transposed (bf16, streamed per column block)
  cosT/sinTs [128, T]  RoPE tables transposed; sinTs rows 0:64 negated
  masks   [4, 128, 512] additive causal masks (0 / -1e9) for diagonal blocks
Matmuls run bf16 (FWL, 1cyc/row); accumulation fp32 in PSUM; softmax
denominators fp32.
"""
import math
import numpy as np
import ml_dtypes
from contextlib import ExitStack

import concourse.bass as bass
import concourse.tile as tile
from concourse import bacc, mybir
from concourse.bass_utils import run_bass_kernel_spmd

F32 = mybir.dt.float32
F32R = mybir.dt.float32r
BF16 = mybir.dt.bfloat16
BF16_NP = ml_dtypes.bfloat16
AF = mybir.ActivationFunctionType

NC_ = 8           # cores
B, T, D = 2, 2048, 2048
H, HD = 16, 128   # heads, head_dim
HPC = H // NC_    # heads per core = 2
TS = 512          # t-super tile
NTS = T // TS     # 4
NCH = D // 128    # 16 contraction chunks
ROWS = T // NC_   # 256 own token rows per batch
SCALE = 1.0 / math.sqrt(HD)
NEG = -1.0e9


def _build_program():
    nc = bacc.Bacc("TRN2", target_bir_lowering=False, debug=False, num_devices=NC_)

    xT_d = nc.dram_tensor("xT", [B, D, T], BF16, kind="ExternalInput")
    wqk_d = nc.dram_tensor("wqk", [D, 4 * 128], BF16, kind="ExternalInput")
    wv_d = nc.dram_tensor("wv", [D, 2 * 128], BF16, kind="ExternalInput")
    wo_d = nc.dram_tensor("wo", [D, D], BF16, kind="ExternalInput")
    cos_d = nc.dram_tensor("cosT", [128, T], F32, kind="ExternalInput")
    sin_d = nc.dram_tensor("sinTs", [128, T], F32, kind="ExternalInput")
    mask_d = nc.dram_tensor("masks", [4, 128, TS], BF16, kind="ExternalInput")
    id_d = nc.dram_tensor("ident", [128, 128], BF16, kind="ExternalInput")
    onem_d = nc.dram_tensor("onem", [128, 128], BF16, kind="ExternalInput")
    bqk_d = nc.dram_tensor("bqk", [128, 4], F32, kind="ExternalInput")
    bv_d = nc.dram_tensor("bv", [1, 2 * 128], F32, kind="ExternalInput")
    bo_d = nc.dram_tensor("bo", [1, D], F32, kind="ExternalInput")
    out_d = nc.dram_tensor("out", [B, ROWS, D], F32, kind="ExternalOutput")

    with tile.TileContext(nc) as tc:
        with ExitStack() as ctx:
            consts = ctx.enter_context(tc.tile_pool(name="consts", bufs=1))
            qkv = ctx.enter_context(tc.tile_pool(name="qkv", bufs=1))
            xp = ctx.enter_context(tc.tile_pool(name="xp", bufs=8))
            prp = ctx.enter_context(tc.tile_pool(name="prp", bufs=3))
            qep = ctx.enter_context(tc.tile_pool(name="qep", bufs=1))
            tmp = ctx.enter_context(tc.tile_pool(name="tmp", bufs=2))
            aosp = ctx.enter_context(tc.tile_pool(name="aosp", bufs=2))
            aogp = ctx.enter_context(tc.tile_pool(name="aogp", bufs=1))
            yop = ctx.enter_context(tc.tile_pool(name="yop", bufs=2))
            dramp = ctx.enter_context(tc.tile_pool(name="dramp", bufs=1, space="DRAM"))

            wqk_t = consts.tile([128, NCH, 4 * 128], BF16)
            wv_t = consts.tile([128, NCH, 2 * 128], BF16)
            wqk_r = wqk_d.ap().rearrange("(c p) e -> p c e", p=128)
            wv_r = wv_d.ap().rearrange("(c p) e -> p c e", p=128)
            wo_r = wo_d.ap().rearrange("(h p) o -> p h o", p=128)
            wo_t = consts.tile([128, H, D], BF16)
            cos_t = consts.tile([128, T], F32)
            nc.scalar.dma_start(out=cos_t, in_=cos_d.ap())
            sin_t = consts.tile([128, T], F32)
            nc.scalar.dma_start(out=sin_t, in_=sin_d.ap())
            mask_t = consts.tile([128, 4, TS], BF16)
            nc.scalar.dma_start(out=mask_t, in_=mask_d.ap().rearrange("m p n -> p m n"))
            id_t = consts.tile([128, 128], BF16)
            nc.scalar.dma_start(out=id_t, in_=id_d.ap())
            onem_t = consts.tile([128, 128], BF16)
            nc.scalar.dma_start(out=onem_t, in_=onem_d.ap())
            bqk_t = consts.tile([128, 4], F32)
            nc.scalar.dma_start(out=bqk_t, in_=bqk_d.ap())
            bv_t = consts.tile([128, 2 * 128], F32)
            nc.gpsimd.dma_start(out=bv_t, in_=bv_d.ap().partition_broadcast(128))
            bo_t = consts.tile([128, D], F32)
            nc.gpsimd.dma_start(out=bo_t, in_=bo_d.ap().partition_broadcast(128))

            def stage1(b, q_t, k_t, v_t):
                with tc.tile_pool(name=f"s1ps{b}", bufs=1, space="PSUM") as s1ps:
                    for ts in range(NTS):
                        qkp = [s1ps.tile([128, TS], F32, tag=f"qkp{j}", name=f"qkp{j}")
                               for j in range(4)]
                        vp = [s1ps.tile([128, 2 * 128], F32, tag=f"vp{tb}", name=f"vp{tb}")[:]
                              for tb in range(4)]
                        for ci in range(NCH):
                            if b == 0 and ts == 0:
                                nc.sync.dma_start(out=wqk_t[:, ci, :], in_=wqk_r[:, ci, :])
                                nc.sync.dma_start(out=wv_t[:, ci, :], in_=wv_r[:, ci, :])
                            elif b == 0 and ts == 1:
                                nc.sync.dma_start(out=wo_t[:, ci, :], in_=wo_r[:, ci, :])
                            xt = xp.tile([128, TS], BF16)
                            nc.sync.dma_start(
                                out=xt,
                                in_=xT_d.ap()[b, ci * 128:(ci + 1) * 128,
                                              ts * TS:(ts + 1) * TS],
                            )
                            st_, sp_ = ci == 0, ci == NCH - 1
                            for j in range(4):
                                nc.tensor.matmul(
                                    qkp[j][:], wqk_t[:, ci, j * 128:(j + 1) * 128], xt[:],
                                    start=st_, stop=sp_)
                            for tb in range(4):
                                nc.tensor.matmul(
                                    vp[tb], xt[:, tb * 128:(tb + 1) * 128],
                                    wv_t[:, ci, :], start=st_, stop=sp_)
                        # fast ACT eviction (frees banks); qs = half-swapped copy
                        qe = [qep.tile([128, TS], F32, tag=f"qe{j}", name=f"qe{j}")
                              for j in range(4)]
                        qs = [qep.tile([128, TS], F32, tag=f"qs{j}", name=f"qs{j}")
                              for j in range(4)]
                        ve = [qep.tile([128, 2 * 128], F32, tag=f"ve{tb}", name=f"ve{tb}")
                              for tb in range(4)]
                        for j in range(4):
                            nc.scalar.activation(qe[j][:], qkp[j][:], AF.Copy)
                            nc.scalar.activation(qs[j][0:64, :], qkp[j][64:128, :], AF.Copy)
                            nc.scalar.activation(qs[j][64:128, :], qkp[j][0:64, :], AF.Copy)
                        for tb in range(4):
                            nc.scalar.activation(ve[tb][:], vp[tb], AF.Copy)
                        # RoPE + bias on DVE (partition-aligned)
                        cs = cos_t[:, ts * TS:(ts + 1) * TS]
                        sn = sin_t[:, ts * TS:(ts + 1) * TS]
                        for j in range(4):
                            dst = (q_t if j < 2 else k_t)[:, j % 2, ts * TS:(ts + 1) * TS]
                            t1 = tmp.tile([128, TS], F32, tag="t1")
                            t2 = tmp.tile([128, TS], F32, tag="t2")
                            nc.vector.tensor_mul(t1[:], qe[j][:], cs)
                            nc.vector.tensor_mul(t2[:], qs[j][:], sn)
                            nc.vector.tensor_add(t1[:], t1[:], t2[:])
                            nc.vector.tensor_scalar_add(dst, t1[:], bqk_t[:, j:j + 1])
                        for tb in range(4):
                            for hh in range(HPC):
                                nc.vector.tensor_add(
                                    v_t[:, ts * 4 + tb, hh, :],
                                    ve[tb][:, hh * 128:(hh + 1) * 128],
                                    bv_t[:, hh * 128:(hh + 1) * 128])

            def attention(b, q_t, k_t, v_t):
                a2a_in = dramp.tile([NC_, HPC, 128, ROWS], BF16, tag=f"a2i{b}",
                                    name=f"a2i{b}")
                a2a_out = dramp.tile([NC_, HPC, 128, ROWS], BF16, tag=f"a2o{b}",
                                     name=f"a2o{b}")
                with tc.tile_pool(name=f"atps{b}", bufs=1, space="PSUM") as atps:
                    for ts in range(NTS):
                        for hh in range(HPC):
                            op = atps.tile([128, TS], F32, tag="op", bufs=2)
                            sm = atps.tile([128, TS], F32, tag="sm", bufs=2)
                            ntk = 4 * (ts + 1)
                            prev = None
                            for tk in range(ntk):
                                stp = atps.tile([128, TS], F32, tag="st", bufs=2)
                                diag = tk >= 4 * ts
                                nc.tensor.matmul(
                                    stp[:], k_t[:, hh, tk * 128:(tk + 1) * 128],
                                    q_t[:, hh, ts * TS:(ts + 1) * TS],
                                    start=True, stop=not diag)
                                if diag:
                                    nc.tensor.matmul(
                                        stp[:], id_t[:], mask_t[:, tk - 4 * ts, :],
                                        start=False, stop=True)
                                pr = prp.tile([128, TS], BF16)
                                nc.scalar.activation(pr[:], stp[:], AF.Exp, scale=SCALE)
                                if prev is not None:
                                    ptk, ppr = prev
                                    nc.tensor.matmul(op[:], v_t[:, ptk, hh, :], ppr[:],
                                                     start=(ptk == 0), stop=False)
                                    nc.tensor.matmul(sm[:], onem_t[:], ppr[:],
                                                     start=(ptk == 0), stop=False)
                                prev = (tk, pr)
                            ptk, ppr = prev
                            nc.tensor.matmul(op[:], v_t[:, ptk, hh, :], ppr[:],
                                             start=(ptk == 0), stop=True)
                            nc.tensor.matmul(sm[:], onem_t[:], ppr[:],
                                             start=(ptk == 0), stop=True)
                            bsb = tmp.tile([128, TS], F32, tag="bsb")
                            with nc.allow_low_precision(reason="softmax recip"):
                                nc.vector.reciprocal_approx_fast(bsb[:], sm[:])
                            aos = aosp.tile([128, TS], BF16, tag="aos")
                            nc.vector.tensor_mul(aos[:], op[:], bsb[:])
                            nc.gpsimd.dma_start(
                                out=a2a_in[2 * ts:2 * ts + 2, hh, :, :].transpose([1, 0, 2]),
                                in_=aos[:].rearrange("d (s q) -> d s q", s=2))
                nc.gpsimd.collective_compute(
                    "AllToAll", mybir.AluOpType.bypass,
                    replica_groups=[list(range(NC_))],
                    ins=[a2a_in.opt()], outs=[a2a_out.opt()])
                return a2a_out

            def outproj(b, a2a_out):
                # a2a_out[src, hh, d, q] == head (2*src+hh) for my ROWS of batch b
                aoG = aogp.tile([128, H, ROWS], BF16, tag="aoG")
                nc.sync.dma_start(
                    out=aoG, in_=a2a_out[:, :, :, :].rearrange("s h d q -> d (s h) q"))
                with tc.tile_pool(name=f"yps{b}", bufs=2, space="PSUM") as yps:
                    for nb in range(D // TS):
                        for tb in range(ROWS // 128):
                            yp = yps.tile([128, TS], F32, tag="yp")
                            for h in range(H):
                                nc.tensor.matmul(
                                    yp[:], aoG[:, h, tb * 128:(tb + 1) * 128],
                                    wo_t[:, h, nb * TS:(nb + 1) * TS],
                                    start=(h == 0), stop=(h == H - 1))
                            yo = yop.tile([128, TS], F32, tag="yo")
                            nc.vector.tensor_add(yo[:], yp[:], bo_t[:, nb * TS:(nb + 1) * TS])
                            nc.scalar.dma_start(
                                out=out_d.ap()[b, tb * 128:(tb + 1) * 128,
                                               nb * TS:(nb + 1) * TS],
                                in_=yo[:])

            a2a_outs = {}
            for b in range(B):
                q_t = qkv.tile([128, HPC, T], BF16, tag="q", name="q_t")   # [d, h, t]
                k_t = qkv.tile([128, HPC, T], BF16, tag="k", name="k_t")
                v_t = qkv.tile([128, NTS * 4, HPC, 128], BF16, tag="v", name="v_t")
                stage1(b, q_t, k_t, v_t)
                if b > 0:
                    outproj(b - 1, a2a_outs[b - 1])
                a2a_outs[b] = attention(b, q_t, k_t, v_t)
            outproj(B - 1, a2a_outs[B - 1])

    nc.compile()
    return nc


_NC_CACHE = None


def _get_program():
    global _NC_CACHE
    if _NC_CACHE is None:
        _NC_CACHE = _build_program()
    return _NC_CACHE


def make_in_maps(x, rope_cos, rope_sin, qkv_w, qkv_b, out_w, out_b):
    x = np.asarray(x, dtype=np.float32)
    qkv_w = np.asarray(qkv_w, dtype=np.float32)
    qkv_b = np.asarray(qkv_b, dtype=np.float32)
    out_w = np.asarray(out_w, dtype=np.float32)
    out_b = np.asarray(out_b, dtype=np.float32)

    xT = np.ascontiguousarray(x.transpose(0, 2, 1)).astype(BF16_NP)  # [B, D, T]
    cosT = np.ascontiguousarray(np.asarray(rope_cos, np.float32)[0, 0].T)  # [128, T]
    sinTs = np.ascontiguousarray(np.asarray(rope_sin, np.float32)[0, 0].T).copy()
    sinTs[0:64, :] *= -1.0

    tk_idx = np.arange(128)[:, None]
    tq_idx = np.arange(TS)[None, :]
    masks = np.stack(
        [np.where(mi * 128 + tk_idx <= tq_idx, 0.0, NEG) for mi in range(4)]
    ).astype(BF16_NP)                                           # [4, 128, TS]
    ident = np.eye(128, dtype=np.float32).astype(BF16_NP)
    onem = np.ones((128, 128), np.float32).astype(BF16_NP)
    wo = np.ascontiguousarray(out_w.T).astype(BF16_NP)          # [D, D]
    bo = out_b.reshape(1, D)

    in_maps = []
    for c in range(NC_):
        h0 = HPC * c
        qr = qkv_w[h0 * 128:(h0 + HPC) * 128]                  # [256, D]
        kr = qkv_w[D + h0 * 128:D + (h0 + HPC) * 128]
        vr = qkv_w[2 * D + h0 * 128:2 * D + (h0 + HPC) * 128]
        wqk = np.ascontiguousarray(np.concatenate([qr, kr], 0).T).astype(BF16_NP)
        wv = np.ascontiguousarray(vr.T).astype(BF16_NP)        # [D, 256]
        bqk = np.stack(
            [qkv_b[h0 * 128:(h0 + 1) * 128],
             qkv_b[(h0 + 1) * 128:(h0 + 2) * 128],
             qkv_b[D + h0 * 128:D + (h0 + 1) * 128],
             qkv_b[D + (h0 + 1) * 128:D + (h0 + 2) * 128]], axis=1)  # [128, 4]
        bv = qkv_b[2 * D + h0 * 128:2 * D + (h0 + HPC) * 128].reshape(1, 256)
        in_maps.append({
            "xT": xT, "wqk": wqk, "wv": wv, "wo": wo,
            "cosT": cosT, "sinTs": sinTs, "masks": masks, "ident": ident,
            "onem": onem,
            "bqk": np.ascontiguousarray(bqk), "bv": np.ascontiguousarray(bv),
            "bo": bo,
        })
    return in_maps


def assemble(results):
    y = np.empty((B, T, D), dtype=np.float32)
    for c in range(NC_):
        y[:, c * ROWS:(c + 1) * ROWS, :] = results[c]["out"]
    return y


def run(inputs, trace=False, trace_cores=None):
    nc = _get_program()
    in_maps = make_in_maps(**inputs)
    res = run_bass_kernel_spmd(
        nc, in_maps, list(range(NC_)), trace=trace,
        trace_cores=trace_cores if trace else None)
    return assemble(res.results), res


def kernel(**inputs) -> np.ndarray:
    y, _ = run(inputs, trace=False)
    return y


# revision 21
# speedup vs baseline: 1.0751x; 1.0751x over previous
"""Causal self-attention (B=2, T=2048, D=2048, H=16, d=128) on 8 TRN2 NeuronCores.

Sharding: head-parallel compute, token-parallel output. Core c owns heads
{2c, 2c+1} for both batches: column-parallel QKV projection, per-head RoPE +
causal attention. The per-head attention outputs are exchanged with a single
AllToAll per batch (each core sends each peer the 256 token rows that peer
owns), after which every core holds all 16 heads for its own 256 rows and
computes the full output projection locally — no reduction collective needed.
Host concatenates the 8 contiguous row shards.

Host-prepped layouts (sharding/layout prep only — all math on device):
  xT      [2, D, T]    x transposed per batch (bf16)
  wqk     [D, 512]     qkv_w rows [q_h0,q_h1,k_h0,k_h1] transposed (bf16)
  wv      [D, 256]     qkv_w v rows transposed (bf16)
  wo      [D, D]       full out_w transposed (bf16, streamed per column block)
  cosT/sinTs [128, T]  RoPE tables transposed; sinTs rows 0:64 negated
  masks   [4, 128, 512] additive causal masks (0 / -1e9) for diagonal blocks
Matmuls run bf16 (FWL, 1cyc/row); accumulation fp32 in PSUM; softmax
denominators fp32.
"""
import math
import numpy as np
import ml_dtypes
from contextlib import ExitStack

import concourse.bass as bass
import concourse.tile as tile
from concourse import bacc, mybir
from concourse.bass_utils import run_bass_kernel_spmd

F32 = mybir.dt.float32
F32R = mybir.dt.float32r
BF16 = mybir.dt.bfloat16
BF16_NP = ml_dtypes.bfloat16
AF = mybir.ActivationFunctionType

NC_ = 8           # cores
B, T, D = 2, 2048, 2048
H, HD = 16, 128   # heads, head_dim
HPC = H // NC_    # heads per core = 2
TS = 512          # t-super tile
NTS = T // TS     # 4
NCH = D // 128    # 16 contraction chunks
ROWS = T // NC_   # 256 own token rows per batch
SCALE = 1.0 / math.sqrt(HD)
NEG = -1.0e9


def _build_program():
    nc = bacc.Bacc("TRN2", target_bir_lowering=False, debug=False, num_devices=NC_)

    xT_d = nc.dram_tensor("xT", [B, D, T], BF16, kind="ExternalInput")
    wqk_d = nc.dram_tensor("wqk", [D, 4 * 128], BF16, kind="ExternalInput")
    wv_d = nc.dram_tensor("wv", [D, 2 * 128], BF16, kind="ExternalInput")
    wo_d = nc.dram_tensor("wo", [D, D], BF16, kind="ExternalInput")
    cos_d = nc.dram_tensor("cosT", [128, T], F32, kind="ExternalInput")
    sin_d = nc.dram_tensor("sinTs", [128, T], F32, kind="ExternalInput")
    mask_d = nc.dram_tensor("masks", [4, 128, TS], BF16, kind="ExternalInput")
    id_d = nc.dram_tensor("ident", [128, 128], BF16, kind="ExternalInput")
    onem_d = nc.dram_tensor("onem", [128, 128], BF16, kind="ExternalInput")
    bqk_d = nc.dram_tensor("bqk", [128, 4], F32, kind="ExternalInput")
    bv_d = nc.dram_tensor("bv", [1, 2 * 128], F32, kind="ExternalInput")
    bo_d = nc.dram_tensor("bo", [1, D], F32, kind="ExternalInput")
    out_d = nc.dram_tensor("out", [B, ROWS, D], F32, kind="ExternalOutput")

    with tile.TileContext(nc) as tc:
        with ExitStack() as ctx:
            consts = ctx.enter_context(tc.tile_pool(name="consts", bufs=1))
            qkv = ctx.enter_context(tc.tile_pool(name="qkv", bufs=1))
            xp = ctx.enter_context(tc.tile_pool(name="xp", bufs=8))
            prp = ctx.enter_context(tc.tile_pool(name="prp", bufs=3))
            qep = ctx.enter_context(tc.tile_pool(name="qep", bufs=1))
            tmp = ctx.enter_context(tc.tile_pool(name="tmp", bufs=2))
            aosp = ctx.enter_context(tc.tile_pool(name="aosp", bufs=2))
            aogp = ctx.enter_context(tc.tile_pool(name="aogp", bufs=1))
            yop = ctx.enter_context(tc.tile_pool(name="yop", bufs=2))
            dramp = ctx.enter_context(tc.tile_pool(name="dramp", bufs=1, space="DRAM"))

            wqk_t = consts.tile([128, NCH, 4 * 128], BF16)
            wv_t = consts.tile([128, NCH, 2 * 128], BF16)
            wqk_r = wqk_d.ap().rearrange("(c p) e -> p c e", p=128)
            wv_r = wv_d.ap().rearrange("(c p) e -> p c e", p=128)
            wo_r = wo_d.ap().rearrange("(h p) o -> p h o", p=128)
            wo_t = consts.tile([128, H, D], BF16)
            cos_t = consts.tile([128, T], F32)
            nc.scalar.dma_start(out=cos_t, in_=cos_d.ap())
            sin_t = consts.tile([128, T], F32)
            nc.scalar.dma_start(out=sin_t, in_=sin_d.ap())
            mask_t = consts.tile([128, 4, TS], BF16)
            nc.scalar.dma_start(out=mask_t, in_=mask_d.ap().rearrange("m p n -> p m n"))
            id_t = consts.tile([128, 128], BF16)
            nc.scalar.dma_start(out=id_t, in_=id_d.ap())
            onem_t = consts.tile([128, 128], BF16)
            nc.scalar.dma_start(out=onem_t, in_=onem_d.ap())
            bqk_t = consts.tile([128, 4], F32)
            nc.scalar.dma_start(out=bqk_t, in_=bqk_d.ap())
            bv_t = consts.tile([128, 2 * 128], F32)
            nc.gpsimd.dma_start(out=bv_t, in_=bv_d.ap().partition_broadcast(128))
            bo_t = consts.tile([128, D], F32)
            nc.gpsimd.dma_start(out=bo_t, in_=bo_d.ap().partition_broadcast(128))

            def stage1(b, q_t, k_t, v_t):
                with tc.tile_pool(name=f"s1ps{b}", bufs=1, space="PSUM") as s1ps:
                    for ts in range(NTS):
                        qkp = [s1ps.tile([128, TS], F32, tag=f"qkp{j}", name=f"qkp{j}")
                               for j in range(4)]
                        vp = [s1ps.tile([128, 2 * 128], F32, tag=f"vp{tb}", name=f"vp{tb}")[:]
                              for tb in range(4)]
                        for ci in range(NCH):
                            if b == 0 and ts == 0:
                                nc.sync.dma_start(out=wqk_t[:, ci, :], in_=wqk_r[:, ci, :])
                                nc.sync.dma_start(out=wv_t[:, ci, :], in_=wv_r[:, ci, :])
                            elif b == 0 and ts == 1:
                                nc.sync.dma_start(out=wo_t[:, ci, :], in_=wo_r[:, ci, :])
                            xt = xp.tile([128, TS], BF16)
                            nc.sync.dma_start(
                                out=xt,
                                in_=xT_d.ap()[b, ci * 128:(ci + 1) * 128,
                                              ts * TS:(ts + 1) * TS],
                            )
                            st_, sp_ = ci == 0, ci == NCH - 1
                            for j in range(4):
                                nc.tensor.matmul(
                                    qkp[j][:], wqk_t[:, ci, j * 128:(j + 1) * 128], xt[:],
                                    start=st_, stop=sp_)
                            for tb in range(4):
                                nc.tensor.matmul(
                                    vp[tb], xt[:, tb * 128:(tb + 1) * 128],
                                    wv_t[:, ci, :], start=st_, stop=sp_)
                        # fast ACT eviction (frees banks); qs = half-swapped copy
                        qe = [qep.tile([128, TS], F32, tag=f"qe{j}", name=f"qe{j}")
                              for j in range(4)]
                        qs = [qep.tile([128, TS], F32, tag=f"qs{j}", name=f"qs{j}")
                              for j in range(4)]
                        ve = [qep.tile([128, 2 * 128], F32, tag=f"ve{tb}", name=f"ve{tb}")
                              for tb in range(4)]
                        for j in range(4):
                            nc.scalar.activation(qe[j][:], qkp[j][:], AF.Copy)
                            nc.scalar.activation(qs[j][0:64, :], qkp[j][64:128, :], AF.Copy)
                            nc.scalar.activation(qs[j][64:128, :], qkp[j][0:64, :], AF.Copy)
                        for tb in range(4):
                            nc.scalar.activation(ve[tb][:], vp[tb], AF.Copy)
                        # RoPE + bias on DVE (partition-aligned)
                        cs = cos_t[:, ts * TS:(ts + 1) * TS]
                        sn = sin_t[:, ts * TS:(ts + 1) * TS]
                        for j in range(4):
                            dst = (q_t if j < 2 else k_t)[:, j % 2, ts * TS:(ts + 1) * TS]
                            t1 = tmp.tile([128, TS], F32, tag="t1")
                            t2 = tmp.tile([128, TS], F32, tag="t2")
                            nc.vector.tensor_mul(t1[:], qe[j][:], cs)
                            nc.vector.tensor_mul(t2[:], qs[j][:], sn)
                            nc.vector.tensor_add(t1[:], t1[:], t2[:])
                            nc.vector.tensor_scalar_add(dst, t1[:], bqk_t[:, j:j + 1])
                        for tb in range(4):
                            for hh in range(HPC):
                                nc.vector.tensor_add(
                                    v_t[:, ts * 4 + tb, hh, :],
                                    ve[tb][:, hh * 128:(hh + 1) * 128],
                                    bv_t[:, hh * 128:(hh + 1) * 128])

            def attention(b, q_t, k_t, v_t):
                # one AllToAll per head so the first exchange overlaps the
                # second head's attention
                a2a_in = [dramp.tile([NC_, 128, ROWS], BF16, tag=f"a2i{b}{hh}",
                                     name=f"a2i{b}{hh}") for hh in range(HPC)]
                a2a_out = [dramp.tile([NC_, 128, ROWS], BF16, tag=f"a2o{b}{hh}",
                                      name=f"a2o{b}{hh}") for hh in range(HPC)]
                with tc.tile_pool(name=f"atps{b}", bufs=1, space="PSUM") as atps:
                    for hh in range(HPC):
                        for ts in range(NTS):
                            op = atps.tile([128, TS], F32, tag="op", bufs=2)
                            sm = atps.tile([128, TS], F32, tag="sm", bufs=2)
                            ntk = 4 * (ts + 1)
                            prev = None
                            for tk in range(ntk):
                                stp = atps.tile([128, TS], F32, tag="st", bufs=2)
                                diag = tk >= 4 * ts
                                nc.tensor.matmul(
                                    stp[:], k_t[:, hh, tk * 128:(tk + 1) * 128],
                                    q_t[:, hh, ts * TS:(ts + 1) * TS],
                                    start=True, stop=not diag)
                                if diag:
                                    nc.tensor.matmul(
                                        stp[:], id_t[:], mask_t[:, tk - 4 * ts, :],
                                        start=False, stop=True)
                                pr = prp.tile([128, TS], BF16)
                                nc.scalar.activation(pr[:], stp[:], AF.Exp, scale=SCALE)
                                if prev is not None:
                                    ptk, ppr = prev
                                    nc.tensor.matmul(op[:], v_t[:, ptk, hh, :], ppr[:],
                                                     start=(ptk == 0), stop=False)
                                    nc.tensor.matmul(sm[:], onem_t[:], ppr[:],
                                                     start=(ptk == 0), stop=False)
                                prev = (tk, pr)
                            ptk, ppr = prev
                            nc.tensor.matmul(op[:], v_t[:, ptk, hh, :], ppr[:],
                                             start=(ptk == 0), stop=True)
                            nc.tensor.matmul(sm[:], onem_t[:], ppr[:],
                                             start=(ptk == 0), stop=True)
                            bsb = tmp.tile([128, TS], F32, tag="bsb")
                            with nc.allow_low_precision(reason="softmax recip"):
                                nc.vector.reciprocal_approx_fast(bsb[:], sm[:])
                            aos = aosp.tile([128, TS], BF16, tag="aos")
                            nc.vector.tensor_mul(aos[:], op[:], bsb[:])
                            nc.gpsimd.dma_start(
                                out=a2a_in[hh][2 * ts:2 * ts + 2, :, :].transpose([1, 0, 2]),
                                in_=aos[:].rearrange("d (s q) -> d s q", s=2))
                        nc.gpsimd.collective_compute(
                            "AllToAll", mybir.AluOpType.bypass,
                            replica_groups=[list(range(NC_))],
                            ins=[a2a_in[hh].opt()], outs=[a2a_out[hh].opt()])
                return a2a_out

            def outproj(b, a2a_out, last=False):
                # a2a_out[hh][src, d, q] == head (2*src+hh) for my ROWS of batch b
                aoG = [aogp.tile([128, NC_, ROWS], BF16, tag=f"aoG{hh}",
                                 name=f"aoG{hh}") for hh in range(HPC)]
                for hh in range(HPC):
                    nc.sync.dma_start(
                        out=aoG[hh],
                        in_=a2a_out[hh][:, :, :].rearrange("s d q -> d s q"))
                nbufs = 1 if last else 2
                with tc.tile_pool(name=f"yps{b}", bufs=nbufs, space="PSUM") as yps:
                    pairs = [(nb, tb) for nb in range(D // TS)
                             for tb in range(ROWS // 128)]
                    yps_tiles = {}
                    # first-half (hh=0 heads) accumulation — can start right
                    # after the first AllToAll lands
                    for (nb, tb) in (pairs if last else []):
                        yp = yps.tile([128, TS], F32, tag=f"yp{nb}{tb}",
                                      name=f"yp{nb}{tb}")
                        yps_tiles[(nb, tb)] = yp
                        for s in range(NC_):
                            nc.tensor.matmul(
                                yp[:], aoG[0][:, s, tb * 128:(tb + 1) * 128],
                                wo_t[:, 2 * s, nb * TS:(nb + 1) * TS],
                                start=(s == 0), stop=False)
                    for (nb, tb) in pairs:
                        if last:
                            yp = yps_tiles[(nb, tb)]
                            hh_list = [1]
                        else:
                            yp = yps.tile([128, TS], F32, tag="yp")
                            hh_list = [0, 1]
                        for hh in hh_list:
                            for s in range(NC_):
                                nc.tensor.matmul(
                                    yp[:], aoG[hh][:, s, tb * 128:(tb + 1) * 128],
                                    wo_t[:, 2 * s + hh, nb * TS:(nb + 1) * TS],
                                    start=(hh == 0 and s == 0),
                                    stop=(hh == 1 and s == NC_ - 1))
                        yo = yop.tile([128, TS], F32, tag="yo")
                        nc.vector.tensor_add(yo[:], yp[:], bo_t[:, nb * TS:(nb + 1) * TS])
                        nc.scalar.dma_start(
                            out=out_d.ap()[b, tb * 128:(tb + 1) * 128,
                                           nb * TS:(nb + 1) * TS],
                            in_=yo[:])

            a2a_outs = {}
            for b in range(B):
                q_t = qkv.tile([128, HPC, T], BF16, tag="q", name="q_t")   # [d, h, t]
                k_t = qkv.tile([128, HPC, T], BF16, tag="k", name="k_t")
                v_t = qkv.tile([128, NTS * 4, HPC, 128], BF16, tag="v", name="v_t")
                stage1(b, q_t, k_t, v_t)
                if b > 0:
                    outproj(b - 1, a2a_outs[b - 1])
                a2a_outs[b] = attention(b, q_t, k_t, v_t)
            outproj(B - 1, a2a_outs[B - 1], last=True)

    nc.compile()
    return nc


_NC_CACHE = None


def _get_program():
    global _NC_CACHE
    if _NC_CACHE is None:
        _NC_CACHE = _build_program()
    return _NC_CACHE


def make_in_maps(x, rope_cos, rope_sin, qkv_w, qkv_b, out_w, out_b):
    x = np.asarray(x, dtype=np.float32)
    qkv_w = np.asarray(qkv_w, dtype=np.float32)
    qkv_b = np.asarray(qkv_b, dtype=np.float32)
    out_w = np.asarray(out_w, dtype=np.float32)
    out_b = np.asarray(out_b, dtype=np.float32)

    xT = np.ascontiguousarray(x.transpose(0, 2, 1)).astype(BF16_NP)  # [B, D, T]
    cosT = np.ascontiguousarray(np.asarray(rope_cos, np.float32)[0, 0].T)  # [128, T]
    sinTs = np.ascontiguousarray(np.asarray(rope_sin, np.float32)[0, 0].T).copy()
    sinTs[0:64, :] *= -1.0

    tk_idx = np.arange(128)[:, None]
    tq_idx = np.arange(TS)[None, :]
    masks = np.stack(
        [np.where(mi * 128 + tk_idx <= tq_idx, 0.0, NEG) for mi in range(4)]
    ).astype(BF16_NP)                                           # [4, 128, TS]
    ident = np.eye(128, dtype=np.float32).astype(BF16_NP)
    onem = np.ones((128, 128), np.float32).astype(BF16_NP)
    wo = np.ascontiguousarray(out_w.T).astype(BF16_NP)          # [D, D]
    bo = out_b.reshape(1, D)

    in_maps = []
    for c in range(NC_):
        h0 = HPC * c
        qr = qkv_w[h0 * 128:(h0 + HPC) * 128]                  # [256, D]
        kr = qkv_w[D + h0 * 128:D + (h0 + HPC) * 128]
        vr = qkv_w[2 * D + h0 * 128:2 * D + (h0 + HPC) * 128]
        wqk = np.ascontiguousarray(np.concatenate([qr, kr], 0).T).astype(BF16_NP)
        wv = np.ascontiguousarray(vr.T).astype(BF16_NP)        # [D, 256]
        bqk = np.stack(
            [qkv_b[h0 * 128:(h0 + 1) * 128],
             qkv_b[(h0 + 1) * 128:(h0 + 2) * 128],
             qkv_b[D + h0 * 128:D + (h0 + 1) * 128],
             qkv_b[D + (h0 + 1) * 128:D + (h0 + 2) * 128]], axis=1)  # [128, 4]
        bv = qkv_b[2 * D + h0 * 128:2 * D + (h0 + HPC) * 128].reshape(1, 256)
        in_maps.append({
            "xT": xT, "wqk": wqk, "wv": wv, "wo": wo,
            "cosT": cosT, "sinTs": sinTs, "masks": masks, "ident": ident,
            "onem": onem,
            "bqk": np.ascontiguousarray(bqk), "bv": np.ascontiguousarray(bv),
            "bo": bo,
        })
    return in_maps


def assemble(results):
    y = np.empty((B, T, D), dtype=np.float32)
    for c in range(NC_):
        y[:, c * ROWS:(c + 1) * ROWS, :] = results[c]["out"]
    return y


def run(inputs, trace=False, trace_cores=None):
    nc = _get_program()
    in_maps = make_in_maps(**inputs)
    res = run_bass_kernel_spmd(
        nc, in_maps, list(range(NC_)), trace=trace,
        trace_cores=trace_cores if trace else None)
    return assemble(res.results), res


def kernel(**inputs) -> np.ndarray:
    y, _ = run(inputs, trace=False)
    return y
